# revision 1
# baseline (speedup 1.0000x reference)
"""MultiHeadAttention TRN2 kernel — head-sharded across 8 NeuronCores.

Sharding: core h owns head h (all 16 batches). Per-head attention is
independent; the big fc weight Wf [512, 262144] is sharded along its
input dim by head (each core reads only its head's [512, 512*64] slice,
pre-permuted on host to matmul-friendly layout). The only cross-core
reduction (summing fc partials [16,512]) is done on host.

Device pipeline per core (head h), all layouts chosen so no on-device
transposes of big tensors are needed:
  - host passes qT/kT (f32) and vT (bf16) pre-transposed per batch
  - qhT/khT = WqT.T @ qT  (f32, PSUM)  -> SBUF f32 [64(dk), 512(l)]
  - vh = vT.T @ WvT (bf16) -> SBUF [128(lk), 64(dv)] x4 chunks
  - scores[lq,lk] = qhT.T @ khT (f32); exp+rowsum via one ACT op;
    normalize; store attn (bf16, host upcasts)
  - scoresT[lk,lq] = khT.T @ qhT computed directly (swapped operands)
    instead of transposing attn; expT feeds attn@V as stationary operand
  - out_att[lq,dv] accumulated in PSUM, normalized by the same rowsum
    reciprocal, PE-transposed to [dv,lq], and scattered into the
    fc-ready layout oT_all[(l%2)*64+d, l//2, b]
  - fc partial: for each K-chunk j (= l-pair), matmul
    lhsT=oT_all[:,j,:] [128,16] x rhs=Wf_perm[j] [128,512] -> acc[16,512]
"""

import os

os.environ.setdefault("MYCRO_LOCAL_CACHE", "1")

from contextlib import ExitStack

import numpy as np
import ml_dtypes

B, L, D, H, DK, DV = 16, 512, 512, 8, 64, 64
TEMP = float(np.sqrt(DK))
NCORES = 8
BF16 = ml_dtypes.bfloat16

_CACHE = {}


def _build_nc():
    import concourse.bass as bass
    import concourse.tile as tile
    from concourse import bacc, mybir
    from concourse.masks import make_identity

    dt = mybir.dt
    f32, bf16 = dt.float32, dt.bfloat16

    nc = bacc.Bacc(
        "TRN2",
        debug=False,
        enable_asserts=False,
        target_bir_lowering=False,
        num_devices=NCORES,
    )

    qT = nc.dram_tensor("qT", [B, D, L], f32, kind="ExternalInput").ap()
    kT = nc.dram_tensor("kT", [B, D, L], f32, kind="ExternalInput").ap()
    vT = nc.dram_tensor("vT", [B, D, L], bf16, kind="ExternalInput").ap()
    wqT = nc.dram_tensor("wqT", [D, DK], f32, kind="ExternalInput").ap()
    wkT = nc.dram_tensor("wkT", [D, DK], f32, kind="ExternalInput").ap()
    wvT = nc.dram_tensor("wvT", [D, DV], bf16, kind="ExternalInput").ap()
    bq2 = nc.dram_tensor("bq2", [DK, 1], f32, kind="ExternalInput").ap()
    bk2 = nc.dram_tensor("bk2", [DK, 1], f32, kind="ExternalInput").ap()
    wf = nc.dram_tensor("wf", [256, 128, 512], bf16, kind="ExternalInput").ap()

    attn_o = nc.dram_tensor("attn_o", [B, L, L], bf16, kind="ExternalOutput").ap()
    out_p = nc.dram_tensor("out_p", [B, 512], f32, kind="ExternalOutput").ap()

    with tile.TileContext(nc) as tc:
        with ExitStack() as ctx:
            consts = ctx.enter_context(tc.tile_pool(name="consts", bufs=1))

            # Constants / persistent SBUF
            wq_s = consts.tile([128, 4, DK], f32, tag="wq")
            wk_s = consts.tile([128, 4, DK], f32, tag="wk")
            wv_s = consts.tile([128, 4, DV], bf16, tag="wv")
            nc.sync.dma_start(out=wq_s, in_=wqT.rearrange("(c p) e -> p c e", p=128))
            nc.sync.dma_start(out=wk_s, in_=wkT.rearrange("(c p) e -> p c e", p=128))
            nc.sync.dma_start(out=wv_s, in_=wvT.rearrange("(c p) e -> p c e", p=128))
            bq_s = consts.tile([DK, 1], f32, tag="bq")
            bk_s = consts.tile([DK, 1], f32, tag="bk")
            nc.sync.dma_start(out=bq_s, in_=bq2)
            nc.sync.dma_start(out=bk_s, in_=bk2)
            ident = consts.tile([128, 128], f32, tag="ident")
            make_identity(nc, ident)

            qh_all = consts.tile([DK, B, L], f32, tag="qh_all")
            kh_all = consts.tile([DK, B, L], f32, tag="kh_all")
            vh_all = consts.tile([128, B, 4, DV], bf16, tag="vh_all")
            oT_all = consts.tile([128, 256, B], bf16, tag="oT_all")
            out_acc = consts.tile([B, 512], f32, tag="out_acc")
            nc.vector.memset(out_acc, 0.0)

            # ---------------- Phase P: projections ----------------
            with ExitStack() as pctx:
                ld = pctx.enter_context(tc.tile_pool(name="ld", bufs=2))
                pp_proj = pctx.enter_context(
                    tc.tile_pool(name="pp_proj", bufs=2, space="PSUM")
                )
                pp_v = pctx.enter_context(
                    tc.tile_pool(name="pp_v", bufs=2, space="PSUM")
                )
                for b in range(B):
                    qt = ld.tile([128, 4, L], f32, tag="qt")
                    kt = ld.tile([128, 4, L], f32, tag="kt")
                    vt = ld.tile([128, 4, L], bf16, tag="vt")
                    nc.sync.dma_start(
                        out=qt, in_=qT[b].rearrange("(c p) l -> p c l", p=128)
                    )
                    nc.sync.dma_start(
                        out=kt, in_=kT[b].rearrange("(c p) l -> p c l", p=128)
                    )
                    nc.sync.dma_start(
                        out=vt, in_=vT[b].rearrange("(c p) l -> p c l", p=128)
                    )
                    psq = pp_proj.tile([DK, L], f32, tag="psq")
                    for c in range(4):
                        nc.tensor.matmul(
                            psq, lhsT=wq_s[:, c, :], rhs=qt[:, c, :],
                            start=(c == 0), stop=(c == 3),
                        )
                    nc.vector.tensor_scalar_add(qh_all[:, b, :], psq, bq_s)
                    psk = pp_proj.tile([DK, L], f32, tag="psq")
                    for c in range(4):
                        nc.tensor.matmul(
                            psk, lhsT=wk_s[:, c, :], rhs=kt[:, c, :],
                            start=(c == 0), stop=(c == 3),
                        )
                    nc.vector.tensor_scalar_add(kh_all[:, b, :], psk, bk_s)
                    for ls in range(4):
                        psv = pp_v.tile([128, DV], f32, tag="psv")
                        for c in range(4):
                            nc.tensor.matmul(
                                psv,
                                lhsT=vt[:, c, ls * 128:(ls + 1) * 128],
                                rhs=wv_s[:, c, :],
                                start=(c == 0), stop=(c == 3),
                            )
                        nc.vector.tensor_copy(vh_all[:, b, ls, :], psv)

            # ---------------- Phase A: attention + fc ----------------
            pp_s = ctx.enter_context(tc.tile_pool(name="pp_s", bufs=2, space="PSUM"))
            pp_st = ctx.enter_context(tc.tile_pool(name="pp_st", bufs=2, space="PSUM"))
            pp_o = ctx.enter_context(tc.tile_pool(name="pp_o", bufs=2, space="PSUM"))
            pp_t = ctx.enter_context(tc.tile_pool(name="pp_t", bufs=1, space="PSUM"))
            pp_fc = ctx.enter_context(tc.tile_pool(name="pp_fc", bufs=1, space="PSUM"))
            sb_e = ctx.enter_context(tc.tile_pool(name="sb_e", bufs=6))
            sb_x = ctx.enter_context(tc.tile_pool(name="sb_x", bufs=3))
            sb_r = ctx.enter_context(tc.tile_pool(name="sb_r", bufs=6))
            wf_pool = ctx.enter_context(tc.tile_pool(name="wf_pool", bufs=6))

            for c in range(4):
                for b in range(B):
                    qsl = qh_all[:, b, c * 128:(c + 1) * 128]
                    # --- path A: attention probabilities (f32) ---
                    ps = pp_s.tile([128, L], f32, tag="ps")
                    nc.tensor.matmul(ps, lhsT=qsl, rhs=kh_all[:, b, :])
                    ex = sb_x.tile([128, L], f32, tag="ex")
                    ssum = sb_r.tile([128, 1], f32, tag="ssum")
                    nc.scalar.activation(
                        ex, ps, func=_EXP(), scale=1.0 / TEMP, accum_out=ssum
                    )
                    r = sb_r.tile([128, 1], f32, tag="r")
                    nc.vector.reciprocal(r, ssum)
                    ab = sb_x.tile([128, L], bf16, tag="ab")
                    nc.vector.tensor_scalar_mul(ab, ex, r)
                    nc.sync.dma_start(
                        out=attn_o[b, c * 128:(c + 1) * 128, :], in_=ab
                    )
                    # --- path B: attention output, transposed scores ---
                    po = pp_o.tile([128, DV], f32, tag="po")
                    for kk in range(4):
                        pst = pp_st.tile([128, 128], f32, tag="pst")
                        nc.tensor.matmul(
                            pst,
                            lhsT=kh_all[:, b, kk * 128:(kk + 1) * 128],
                            rhs=qsl,
                        )
                        eT = sb_e.tile([128, 128], bf16, tag="eT")
                        nc.scalar.activation(eT, pst, func=_EXP(), scale=1.0 / TEMP)
                        nc.tensor.matmul(
                            po, lhsT=eT, rhs=vh_all[:, b, kk, :],
                            start=(kk == 0), stop=(kk == 3),
                        )
                    oa = sb_e.tile([128, DV], f32, tag="oa")
                    nc.vector.tensor_scalar_mul(oa, po, r)
                    pt = pp_t.tile([DV, 128], f32, tag="pt")
                    nc.tensor.transpose(pt, oa, ident)
                    ptv = pt.rearrange("p (m x) -> p m x", x=2)
                    nc.vector.tensor_copy(
                        oT_all[0:64, 64 * c:64 * (c + 1), b], ptv[:, :, 0]
                    )
                    nc.vector.tensor_copy(
                        oT_all[64:128, 64 * c:64 * (c + 1), b], ptv[:, :, 1]
                    )
                # --- fc for this l-chunk: j in [64c, 64c+64) ---
                acc = pp_fc.tile([B, 512], f32, tag="acc")
                for jj in range(0, 64, 4):
                    wt = wf_pool.tile([128, 4, 512], bf16, tag="wt")
                    j0 = 64 * c + jj
                    nc.sync.dma_start(
                        out=wt, in_=wf[j0:j0 + 4].rearrange("j p o -> p j o")
                    )
                    for js in range(4):
                        nc.tensor.matmul(
                            acc,
                            lhsT=oT_all[:, j0 + js, :],
                            rhs=wt[:, js, :],
                            start=(jj == 0 and js == 0),
                            stop=(jj == 60 and js == 3),
                            skip_group_check=True,
                        )
                nc.vector.tensor_add(out_acc, out_acc, acc)

            nc.sync.dma_start(out=out_p, in_=out_acc)

    nc.compile()
    return nc


def _EXP():
    from concourse import mybir

    return mybir.ActivationFunctionType.Exp


def _get_nc():
    if "nc" not in _CACHE:
        _CACHE["nc"] = _build_nc()
    return _CACHE["nc"]


def make_in_maps(q, k, v, Wq, bq, Wk, bk, Wv, bv, Wf, bf):
    """Host-side sharding/layout prep. Returns per-core input dicts."""
    q, k, v = (np.asarray(x, np.float32) for x in (q, k, v))
    qT = np.ascontiguousarray(q.transpose(0, 2, 1))
    kT = np.ascontiguousarray(k.transpose(0, 2, 1))
    vTb = v.transpose(0, 2, 1).astype(BF16)
    Wfv = np.asarray(Wf, np.float32).reshape(512, L, H, DV)
    in_maps = []
    for h in range(H):
        sl = slice(h * DK, (h + 1) * DK)
        wf_h = (
            Wfv[:, :, h, :].transpose(1, 2, 0).astype(BF16).reshape(256, 128, 512)
        )
        in_maps.append(
            {
                "qT": qT,
                "kT": kT,
                "vT": vTb,
                "wqT": np.ascontiguousarray(np.asarray(Wq, np.float32)[sl].T),
                "wkT": np.ascontiguousarray(np.asarray(Wk, np.float32)[sl].T),
                "wvT": np.asarray(Wv, np.float32)[sl].T.astype(BF16),
                "bq2": np.asarray(bq, np.float32)[sl].reshape(DK, 1).copy(),
                "bk2": np.asarray(bk, np.float32)[sl].reshape(DK, 1).copy(),
                "wf": wf_h,
            }
        )
    return in_maps


def assemble_outputs(results, bv, bf, Wf):
    """results: per-core dicts with attn_o [B,L,L] bf16 and out_p [B,512]."""
    attn_flat = np.concatenate(
        [np.asarray(results[h]["attn_o"], np.float32)[None] for h in range(H)], 0
    ).reshape(H * B, L, L)
    output = np.zeros((B, 512), np.float32)
    for h in range(H):
        output += np.asarray(results[h]["out_p"], np.float32)
    bv = np.asarray(bv, np.float32)
    if np.any(bv):
        Wfs = np.asarray(Wf, np.float32).reshape(512, L, H, DV).sum(axis=1)
        output += np.einsum("ohd,hd->o", Wfs, bv.reshape(H, DV))[None, :]
    output += np.asarray(bf, np.float32)[None, :]
    return output, attn_flat


def kernel(q, k, v, Wq, bq, Wk, bk, Wv, bv, Wf, bf):
    from concourse.bass_utils import run_bass_kernel_spmd

    nc = _get_nc()
    in_maps = make_in_maps(q, k, v, Wq, bq, Wk, bk, Wv, bv, Wf, bf)
    res = run_bass_kernel_spmd(nc, in_maps, core_ids=list(range(NCORES)))
    output, attn_flat = assemble_outputs(res.results, bv, bf, Wf)
    return output, attn_flat


# revision 13
# speedup vs baseline: 13.4365x; 13.4365x over previous
"""MultiHeadAttention TRN2 kernel — head-sharded across 8 NeuronCores.

Sharding: core h owns head h (all 16 batches). Per-head attention is
independent; the big fc weight Wf [512, 262144] is sharded along its
input dim by head (each core reads only its head's [512, 512*64] slice,
pre-permuted on host to matmul-friendly layout). The only cross-core
reduction (summing fc partials [16,512]) is done on host.

Precision: fp16 for the q/k projection + scores path (fp32 matmuls are
4x slower on the PE and double the DMA); f32 softmax; bf16 for expT /
vh / out_att / Wf. Measured end-to-end: out ~4e-3, attn ~2.4e-3
absmax-rel.

Device pipeline per core (head h), all layouts chosen so no on-device
transposes of big tensors are needed:
  - host passes qT/kT (f16) and vT (bf16) pre-transposed per batch
  - qhT/khT = WqT.T @ qT -> SBUF f16 [64(dk), 512(l)]
  - vh = vT.T @ WvT (bf16) -> SBUF [128(lk), 64(dv)] x4 chunks
  - scores[lq,lk] = qhT.T @ khT (f16 in, f32 PSUM); exp+rowsum in one
    ACT op; normalize; store attn (bf16, host upcasts)
  - scoresT[lk,lq] = khT.T @ qhT computed directly (swapped operands)
    instead of transposing attn; all 4 lk-chunks land in one PSUM bank
    so one ACT op does all of exp(scoresT); expT feeds attn@V as the
    stationary operand
  - out_att[lq,dv] accumulated in PSUM, normalized by the same rowsum
    reciprocal, PE-transposed to [dv,lq], and scattered into the
    fc-ready layout oT_c[(l%2)*64+d, l//2 within chunk, b]
  - fc partial: for each K-chunk j (= l-pair), matmul
    lhsT=oT_c[:,jj,:] [128,16] x rhs=Wf_perm[j] [128,512] -> acc[16,512]

DMA issue is split across both HWDGE sequencers (SP for loads, ACT for
wf/attn) and batched (qkv 2 batches, wf 8 j-chunks, attn 4 batches per
dma_start) to keep per-DMA sequencer overhead (~0.6us each) off the
critical path.
"""

import os

os.environ.setdefault("MYCRO_LOCAL_CACHE", "1")

from contextlib import ExitStack

import numpy as np
import ml_dtypes

B, L, D, H, DK, DV = 16, 512, 512, 8, 64, 64
TEMP = float(np.sqrt(DK))
NCORES = 8
BF16 = ml_dtypes.bfloat16

_CACHE = {}


def _build_nc(niter=1, ld_bufs=2, wf_bufs=6, e_bufs=4, x_bufs=3, ab_bufs=3,
              r_bufs=8, s_bufs=2, st_bufs=2, o_bufs=2, attn_store_engine="scalar"):
    import concourse.bass as bass
    import concourse.tile as tile
    from concourse import bacc, mybir
    from concourse.masks import make_identity

    dt = mybir.dt
    f32, bf16, f16 = dt.float32, dt.bfloat16, dt.float16
    EXP = mybir.ActivationFunctionType.Exp

    nc = bacc.Bacc(
        "TRN2",
        debug=False,
        enable_asserts=False,
        target_bir_lowering=False,
        num_devices=NCORES,
    )

    qT = nc.dram_tensor("qT", [B, D, L], f16, kind="ExternalInput").ap()
    kT = nc.dram_tensor("kT", [B, D, L], f16, kind="ExternalInput").ap()
    vT = nc.dram_tensor("vT", [B, D, L], bf16, kind="ExternalInput").ap()
    wqT = nc.dram_tensor("wqT", [D, DK], f16, kind="ExternalInput").ap()
    wkT = nc.dram_tensor("wkT", [D, DK], f16, kind="ExternalInput").ap()
    wvT = nc.dram_tensor("wvT", [D, DV], bf16, kind="ExternalInput").ap()
    bq2 = nc.dram_tensor("bq2", [DK, 1], f32, kind="ExternalInput").ap()
    bk2 = nc.dram_tensor("bk2", [DK, 1], f32, kind="ExternalInput").ap()
    wf = nc.dram_tensor("wf", [256, 128, 512], bf16, kind="ExternalInput").ap()

    attn_o = nc.dram_tensor("attn_o", [B, L, L], bf16, kind="ExternalOutput").ap()
    out_p = nc.dram_tensor("out_p", [B, 512], f32, kind="ExternalOutput").ap()

    with tile.TileContext(nc) as tc:
        with ExitStack() as ctx:
            consts = ctx.enter_context(tc.tile_pool(name="consts", bufs=1))

            wq_s = consts.tile([128, 4, DK], f16, tag="wq")
            wk_s = consts.tile([128, 4, DK], f16, tag="wk")
            wv_s = consts.tile([128, 4, DV], bf16, tag="wv")
            nc.sync.dma_start(out=wq_s, in_=wqT.rearrange("(c p) e -> p c e", p=128))
            nc.sync.dma_start(out=wk_s, in_=wkT.rearrange("(c p) e -> p c e", p=128))
            nc.sync.dma_start(out=wv_s, in_=wvT.rearrange("(c p) e -> p c e", p=128))
            bq_s = consts.tile([DK, 1], f32, tag="bq")
            bk_s = consts.tile([DK, 1], f32, tag="bk")
            nc.sync.dma_start(out=bq_s, in_=bq2)
            nc.sync.dma_start(out=bk_s, in_=bk2)
            ident = consts.tile([128, 128], f32, tag="ident")
            make_identity(nc, ident)

            qh_all = consts.tile([DK, B, L], f16, tag="qh_all")
            kh_all = consts.tile([DK, B, L], f16, tag="kh_all")
            vh_all = consts.tile([128, B, 4, DV], bf16, tag="vh_all")
            # fc-ready attention output, one tile per l-chunk c so fc reads
            # of chunk c never conflict with writes of chunk c+1
            oT_c = [
                consts.tile([128, 64, B], bf16, tag=f"oT{c}", name=f"oT{c}")
                for c in range(4)
            ]
            out_acc = consts.tile([B, 512], f32, tag="out_acc")

            for _it in range(niter):
                # ---------------- Phase P: projections ----------------
                with ExitStack() as pctx:
                    ld = pctx.enter_context(tc.tile_pool(name="ld", bufs=ld_bufs))
                    pp_proj = pctx.enter_context(
                        tc.tile_pool(name="pp_proj", bufs=2, space="PSUM")
                    )
                    pp_v = pctx.enter_context(
                        tc.tile_pool(name="pp_v", bufs=2, space="PSUM")
                    )
                    for b0 in range(0, B, 2):
                        qt = ld.tile([128, 2, 4, L], f16, tag="qt")
                        kt = ld.tile([128, 2, 4, L], f16, tag="kt")
                        vt = ld.tile([128, 2, 4, L], bf16, tag="vt")
                        nc.sync.dma_start(
                            out=qt,
                            in_=qT[b0:b0 + 2].rearrange("b (c p) l -> p b c l", p=128),
                        )
                        nc.sync.dma_start(
                            out=kt,
                            in_=kT[b0:b0 + 2].rearrange("b (c p) l -> p b c l", p=128),
                        )
                        nc.sync.dma_start(
                            out=vt,
                            in_=vT[b0:b0 + 2].rearrange("b (c p) l -> p b c l", p=128),
                        )
                        for bb in range(2):
                            b = b0 + bb
                            psq = pp_proj.tile([DK, L], f32, tag="psq")
                            for c in range(4):
                                nc.tensor.matmul(
                                    psq, lhsT=wq_s[:, c, :], rhs=qt[:, bb, c, :],
                                    start=(c == 0), stop=(c == 3),
                                )
                            nc.vector.tensor_scalar_add(qh_all[:, b, :], psq, bq_s)
                            psk = pp_proj.tile([DK, L], f32, tag="psq")
                            for c in range(4):
                                nc.tensor.matmul(
                                    psk, lhsT=wk_s[:, c, :], rhs=kt[:, bb, c, :],
                                    start=(c == 0), stop=(c == 3),
                                )
                            nc.vector.tensor_scalar_add(kh_all[:, b, :], psk, bk_s)
                            for ls in range(4):
                                psv = pp_v.tile([128, DV], f32, tag="psv")
                                for c in range(4):
                                    nc.tensor.matmul(
                                        psv,
                                        lhsT=vt[:, bb, c, ls * 128:(ls + 1) * 128],
                                        rhs=wv_s[:, c, :],
                                        start=(c == 0), stop=(c == 3),
                                    )
                                nc.vector.tensor_copy(vh_all[:, b, ls, :], psv)

                # ---------------- Phase A: attention + fc ----------------
                with ExitStack() as actx:
                    pp_s = actx.enter_context(
                        tc.tile_pool(name="pp_s", bufs=s_bufs, space="PSUM"))
                    pp_st = actx.enter_context(
                        tc.tile_pool(name="pp_st", bufs=st_bufs, space="PSUM"))
                    pp_o = actx.enter_context(
                        tc.tile_pool(name="pp_o", bufs=o_bufs, space="PSUM"))
                    pp_t = actx.enter_context(
                        tc.tile_pool(name="pp_t", bufs=1, space="PSUM"))
                    pp_fc = actx.enter_context(
                        tc.tile_pool(name="pp_fc", bufs=1, space="PSUM"))
                    sb_e = actx.enter_context(tc.tile_pool(name="sb_e", bufs=e_bufs))
                    sb_x = actx.enter_context(tc.tile_pool(name="sb_x", bufs=x_bufs))
                    sb_ab = actx.enter_context(tc.tile_pool(name="sb_ab", bufs=ab_bufs))
                    sb_r = actx.enter_context(tc.tile_pool(name="sb_r", bufs=r_bufs))
                    wf_pool = actx.enter_context(tc.tile_pool(name="wf_pool", bufs=wf_bufs))

                    # fc accumulator: one PSUM accumulation group spanning all
                    # 256 fc matmuls. fc for chunk c-1 is software-pipelined
                    # into chunk c's attention loop (4 matmuls per batch) so
                    # PE/DMA stream fc while ACT/DVE run attention.
                    acc = pp_fc.tile([B, 512], f32, tag="acc")
                    wt = None

                    def fc_piece(cp, jj, last=False):
                        nonlocal wt
                        if jj % 8 == 0:
                            wt = wf_pool.tile([128, 8, 512], bf16, tag="wt",
                                              name="wt")
                            j0 = 64 * cp + jj
                            nc.sync.dma_start(
                                out=wt,
                                in_=wf[j0:j0 + 8].rearrange("j p o -> p j o"),
                            )
                        nc.tensor.matmul(
                            acc,
                            lhsT=oT_c[cp][:, jj, :],
                            rhs=wt[:, jj % 8, :],
                            start=(cp == 0 and jj == 0),
                            stop=last,
                            skip_group_check=True,
                        )

                    for c in range(4):
                        for b in range(B):
                            if b % 4 == 0:
                                ab4 = sb_ab.tile([128, 4, L], bf16, tag="ab4")
                            qsl = qh_all[:, b, c * 128:(c + 1) * 128]
                            # --- path A: attention probabilities ---
                            ps = pp_s.tile([128, L], f32, tag="ps")
                            nc.tensor.matmul(ps, lhsT=qsl, rhs=kh_all[:, b, :])
                            ex = sb_x.tile([128, L], f32, tag="ex")
                            ssum = sb_r.tile([128, 1], f32, tag="ssum")
                            nc.scalar.activation(
                                ex, ps, func=EXP, scale=1.0 / TEMP, accum_out=ssum
                            )
                            r = sb_r.tile([128, 1], f32, tag="r")
                            nc.vector.reciprocal(r, ssum)
                            nc.vector.tensor_scalar_mul(ab4[:, b % 4, :], ex, r)
                            if b % 4 == 3:
                                _store = getattr(nc, attn_store_engine)
                                _store.dma_start(
                                    out=attn_o[
                                        b - 3:b + 1, c * 128:(c + 1) * 128, :
                                    ].rearrange("b p l -> p b l"),
                                    in_=ab4,
                                )
                            # --- path B: attention output via scoresT ---
                            pst4 = pp_st.tile([128, 4, 128], f32, tag="pst4")
                            for kk in range(4):
                                nc.tensor.matmul(
                                    pst4[:, kk, :],
                                    lhsT=kh_all[:, b, kk * 128:(kk + 1) * 128],
                                    rhs=qsl,
                                )
                            eT4 = sb_e.tile([128, 4, 128], bf16, tag="eT4")
                            nc.scalar.activation(eT4, pst4, func=EXP, scale=1.0 / TEMP)
                            po = pp_o.tile([128, DV], f32, tag="po")
                            for kk in range(4):
                                nc.tensor.matmul(
                                    po, lhsT=eT4[:, kk, :], rhs=vh_all[:, b, kk, :],
                                    start=(kk == 0), stop=(kk == 3),
                                )
                            oa = sb_e.tile([128, DV], f32, tag="oa")
                            nc.vector.tensor_scalar_mul(oa, po, r)
                            pt = pp_t.tile([DV, 128], f32, tag="pt")
                            nc.tensor.transpose(pt, oa, ident)
                            ptv = pt.rearrange("p (m x) -> p m x", x=2)
                            nc.vector.tensor_copy(oT_c[c][0:64, :, b], ptv[:, :, 0])
                            nc.vector.tensor_copy(oT_c[c][64:128, :, b], ptv[:, :, 1])
                            if c >= 1:
                                for t in range(4):
                                    fc_piece(c - 1, b * 4 + t)
                    # tail: fc for the last l-chunk
                    for jj in range(64):
                        fc_piece(3, jj, last=(jj == 63))
                    nc.vector.tensor_copy(out_acc, acc)

                nc.sync.dma_start(out=out_p, in_=out_acc)

    nc.compile()
    return nc


def _get_nc():
    if "nc" not in _CACHE:
        _CACHE["nc"] = _build_nc()
    return _CACHE["nc"]


def make_in_maps(q, k, v, Wq, bq, Wk, bk, Wv, bv, Wf, bf):
    """Host-side sharding/layout prep. Returns per-core input dicts."""
    q, k, v = (np.asarray(x, np.float32) for x in (q, k, v))
    qT = np.ascontiguousarray(q.transpose(0, 2, 1)).astype(np.float16)
    kT = np.ascontiguousarray(k.transpose(0, 2, 1)).astype(np.float16)
    vTb = v.transpose(0, 2, 1).astype(BF16)
    Wfv = np.asarray(Wf, np.float32).reshape(512, L, H, DV)
    in_maps = []
    for h in range(H):
        sl = slice(h * DK, (h + 1) * DK)
        wf_h = (
            Wfv[:, :, h, :].transpose(1, 2, 0).astype(BF16).reshape(256, 128, 512)
        )
        in_maps.append(
            {
                "qT": qT,
                "kT": kT,
                "vT": vTb,
                "wqT": np.asarray(Wq, np.float32)[sl].T.astype(np.float16),
                "wkT": np.asarray(Wk, np.float32)[sl].T.astype(np.float16),
                "wvT": np.asarray(Wv, np.float32)[sl].T.astype(BF16),
                "bq2": np.asarray(bq, np.float32)[sl].reshape(DK, 1).copy(),
                "bk2": np.asarray(bk, np.float32)[sl].reshape(DK, 1).copy(),
                "wf": wf_h,
            }
        )
    return in_maps


def assemble_outputs(results, bv, bf, Wf):
    """results: per-core dicts with attn_o [B,L,L] bf16 and out_p [B,512]."""
    attn_flat = np.concatenate(
        [np.asarray(results[h]["attn_o"], np.float32)[None] for h in range(H)], 0
    ).reshape(H * B, L, L)
    output = np.zeros((B, 512), np.float32)
    for h in range(H):
        output += np.asarray(results[h]["out_p"], np.float32)
    bv = np.asarray(bv, np.float32)
    if np.any(bv):
        Wfs = np.asarray(Wf, np.float32).reshape(512, L, H, DV).sum(axis=1)
        output += np.einsum("ohd,hd->o", Wfs, bv.reshape(H, DV))[None, :]
    output += np.asarray(bf, np.float32)[None, :]
    return output, attn_flat


def kernel(q, k, v, Wq, bq, Wk, bk, Wv, bv, Wf, bf):
    from concourse.bass_utils import run_bass_kernel_spmd

    nc = _get_nc()
    in_maps = make_in_maps(q, k, v, Wq, bq, Wk, bk, Wv, bv, Wf, bf)
    res = run_bass_kernel_spmd(nc, in_maps, core_ids=list(range(NCORES)))
    output, attn_flat = assemble_outputs(res.results, bv, bf, Wf)
    return output, attn_flat


# revision 17
# speedup vs baseline: 27.6674x; 2.0591x over previous
"""MultiHeadAttention TRN2 kernel — head-sharded across 8 NeuronCores.

Sharding: core h owns head h (all 16 batches). Per-head attention is
independent; the big fc weight Wf [512, 262144] is sharded along its
input dim by head (each core reads only its head's [512, 512*64] slice,
pre-permuted on host to matmul-friendly layout). The only cross-core
reduction (summing fc partials [16,512]) is done on host.

Precision: fp16 everywhere on the streaming tensors (same bytes as
bf16, 10 vs 8 mantissa bits; fp32 matmuls are 4x slower on the PE and
double the DMA); f32 softmax/PSUM. exp computed with a softmax-invariant
-2.0 shift so exp values stay well under fp16 max. Measured end-to-end:
out ~1.3e-3, attn ~1e-3 absmax-rel.

Device pipeline per core (head h), all layouts chosen so no on-device
transposes of big tensors are needed:
  - host passes qT/kT/vT (f16) pre-transposed per batch
  - qhT/khT = WqT.T @ qT -> SBUF f16 [64(dk), 512(l)]
  - vh = vT.T @ WvT (f16) -> SBUF [128(lk), 64(dv)] x4 chunks
  - scores[lq,lk] = qhT.T @ khT (f16 in, f32 PSUM); exp+rowsum in one
    ACT op; normalize; store attn (bf16, host upcasts)
  - scoresT[lk,lq] = khT.T @ qhT computed directly (swapped operands)
    instead of transposing attn; all 4 lk-chunks land in one PSUM bank
    so one ACT op does all of exp(scoresT); expT feeds attn@V as the
    stationary operand
  - out_att[lq,dv] accumulated in PSUM, normalized by the same rowsum
    reciprocal, PE-transposed to [dv,lq], and scattered into the
    fc-ready layout oT_c[(l%2)*64+d, l//2 within chunk, b]
  - fc partial: for each K-chunk j (= l-pair), matmul
    lhsT=oT_c[:,jj,:] [128,16] x rhs=Wf_perm[j] [128,512] -> acc[16,512]

DMA issue is split across both HWDGE sequencers (SP for loads, ACT for
wf/attn) and batched (qkv 2 batches, wf 8 j-chunks, attn 4 batches per
dma_start) to keep per-DMA sequencer overhead (~0.6us each) off the
critical path.
"""

import os

os.environ.setdefault("MYCRO_LOCAL_CACHE", "1")

from contextlib import ExitStack

import numpy as np
import ml_dtypes

B, L, D, H, DK, DV = 16, 512, 512, 8, 64, 64
TEMP = float(np.sqrt(DK))
NCORES = 8
BF16 = ml_dtypes.bfloat16

_CACHE = {}


def _build_nc(niter=1, ld_bufs=2, wf_bufs=6, e_bufs=4, x_bufs=3, ab_bufs=4,
              r_bufs=10, s_bufs=2, st_bufs=2, o_bufs=2, attn_store_engine="scalar"):
    import concourse.bass as bass
    import concourse.tile as tile
    from concourse import bacc, mybir
    from concourse.masks import make_identity

    dt = mybir.dt
    f32, bf16, f16 = dt.float32, dt.bfloat16, dt.float16
    EXP = mybir.ActivationFunctionType.Exp
    EXP_SHIFT = 2.0

    nc = bacc.Bacc(
        "TRN2",
        debug=False,
        enable_asserts=False,
        target_bir_lowering=False,
        num_devices=NCORES,
    )

    qT = nc.dram_tensor("qT", [B, D, L], f16, kind="ExternalInput").ap()
    kT = nc.dram_tensor("kT", [B, D, L], f16, kind="ExternalInput").ap()
    vT = nc.dram_tensor("vT", [B, D, L], f16, kind="ExternalInput").ap()
    wqT = nc.dram_tensor("wqT", [D, DK], f16, kind="ExternalInput").ap()
    wkT = nc.dram_tensor("wkT", [D, DK], f16, kind="ExternalInput").ap()
    wvT = nc.dram_tensor("wvT", [D, DV], f16, kind="ExternalInput").ap()
    bq2 = nc.dram_tensor("bq2", [DK, 1], f32, kind="ExternalInput").ap()
    bk2 = nc.dram_tensor("bk2", [DK, 1], f32, kind="ExternalInput").ap()
    wf = nc.dram_tensor("wf", [256, 128, 512], f16, kind="ExternalInput").ap()

    attn_o = nc.dram_tensor("attn_o", [B, L, L], f16, kind="ExternalOutput").ap()
    out_p = nc.dram_tensor("out_p", [B, 512], f32, kind="ExternalOutput").ap()

    with tile.TileContext(nc) as tc:
        with ExitStack() as ctx:
            consts = ctx.enter_context(tc.tile_pool(name="consts", bufs=1))

            wq_s = consts.tile([128, 4, DK], f16, tag="wq")
            wk_s = consts.tile([128, 4, DK], f16, tag="wk")
            wv_s = consts.tile([128, 4, DV], f16, tag="wv")
            nc.sync.dma_start(out=wq_s, in_=wqT.rearrange("(c p) e -> p c e", p=128))
            nc.sync.dma_start(out=wk_s, in_=wkT.rearrange("(c p) e -> p c e", p=128))
            nc.sync.dma_start(out=wv_s, in_=wvT.rearrange("(c p) e -> p c e", p=128))
            bq_s = consts.tile([DK, 1], f32, tag="bq")
            bk_s = consts.tile([DK, 1], f32, tag="bk")
            nc.sync.dma_start(out=bq_s, in_=bq2)
            nc.sync.dma_start(out=bk_s, in_=bk2)
            ident = consts.tile([128, 128], f32, tag="ident")
            make_identity(nc, ident)
            shift_s = consts.tile([128, 1], f32, tag="shift")
            nc.vector.memset(shift_s, -EXP_SHIFT)

            qh_all = consts.tile([DK, B, L], f16, tag="qh_all")
            kh_all = consts.tile([DK, B, L], f16, tag="kh_all")
            vh_all = consts.tile([128, B, 4, DV], f16, tag="vh_all")
            # fc-ready attention output, one tile per l-chunk c so fc reads
            # of chunk c never conflict with writes of chunk c+1
            oT_c = [
                consts.tile([128, 64, B], f16, tag=f"oT{c}", name=f"oT{c}")
                for c in range(4)
            ]
            out_acc = consts.tile([B, 512], f32, tag="out_acc")

            for _it in range(niter):
                # ---------------- Phase P: projections ----------------
                with ExitStack() as pctx:
                    ld = pctx.enter_context(tc.tile_pool(name="ld", bufs=ld_bufs))
                    pp_proj = pctx.enter_context(
                        tc.tile_pool(name="pp_proj", bufs=2, space="PSUM")
                    )
                    pp_v = pctx.enter_context(
                        tc.tile_pool(name="pp_v", bufs=2, space="PSUM")
                    )
                    for b0 in range(0, B, 2):
                        qt = ld.tile([128, 2, 4, L], f16, tag="qt")
                        kt = ld.tile([128, 2, 4, L], f16, tag="kt")
                        vt = ld.tile([128, 2, 4, L], f16, tag="vt")
                        nc.sync.dma_start(
                            out=qt,
                            in_=qT[b0:b0 + 2].rearrange("b (c p) l -> p b c l", p=128),
                        )
                        nc.sync.dma_start(
                            out=kt,
                            in_=kT[b0:b0 + 2].rearrange("b (c p) l -> p b c l", p=128),
                        )
                        nc.sync.dma_start(
                            out=vt,
                            in_=vT[b0:b0 + 2].rearrange("b (c p) l -> p b c l", p=128),
                        )
                        for bb in range(2):
                            b = b0 + bb
                            psq = pp_proj.tile([DK, L], f32, tag="psq")
                            for c in range(4):
                                nc.tensor.matmul(
                                    psq, lhsT=wq_s[:, c, :], rhs=qt[:, bb, c, :],
                                    start=(c == 0), stop=(c == 3),
                                )
                            nc.vector.tensor_scalar_add(qh_all[:, b, :], psq, bq_s)
                            psk = pp_proj.tile([DK, L], f32, tag="psq")
                            for c in range(4):
                                nc.tensor.matmul(
                                    psk, lhsT=wk_s[:, c, :], rhs=kt[:, bb, c, :],
                                    start=(c == 0), stop=(c == 3),
                                )
                            nc.vector.tensor_scalar_add(kh_all[:, b, :], psk, bk_s)
                            for ls in range(4):
                                psv = pp_v.tile([128, DV], f32, tag="psv")
                                for c in range(4):
                                    nc.tensor.matmul(
                                        psv,
                                        lhsT=vt[:, bb, c, ls * 128:(ls + 1) * 128],
                                        rhs=wv_s[:, c, :],
                                        start=(c == 0), stop=(c == 3),
                                    )
                                nc.vector.tensor_copy(vh_all[:, b, ls, :], psv)

                # ---------------- Phase A: attention + fc ----------------
                with ExitStack() as actx:
                    pp_s = actx.enter_context(
                        tc.tile_pool(name="pp_s", bufs=s_bufs, space="PSUM"))
                    pp_st = actx.enter_context(
                        tc.tile_pool(name="pp_st", bufs=st_bufs, space="PSUM"))
                    pp_o = actx.enter_context(
                        tc.tile_pool(name="pp_o", bufs=o_bufs, space="PSUM"))
                    pp_t = actx.enter_context(
                        tc.tile_pool(name="pp_t", bufs=1, space="PSUM"))
                    pp_fc = actx.enter_context(
                        tc.tile_pool(name="pp_fc", bufs=1, space="PSUM"))
                    sb_e = actx.enter_context(tc.tile_pool(name="sb_e", bufs=e_bufs))
                    sb_x = actx.enter_context(tc.tile_pool(name="sb_x", bufs=x_bufs))
                    sb_ab = actx.enter_context(tc.tile_pool(name="sb_ab", bufs=ab_bufs))
                    sb_r = actx.enter_context(tc.tile_pool(name="sb_r", bufs=r_bufs))
                    wf_pool = actx.enter_context(tc.tile_pool(name="wf_pool", bufs=wf_bufs))

                    # fc accumulator: one PSUM accumulation group spanning all
                    # 256 fc matmuls. fc for chunk c-1 is software-pipelined
                    # into chunk c's attention loop (4 matmuls per batch) so
                    # PE/DMA stream fc while ACT/DVE run attention.
                    acc = pp_fc.tile([B, 512], f32, tag="acc")
                    wt = None

                    def fc_piece(cp, jj, last=False):
                        nonlocal wt
                        if jj % 8 == 0:
                            wt = wf_pool.tile([128, 8, 512], f16, tag="wt",
                                              name="wt")
                            j0 = 64 * cp + jj
                            nc.sync.dma_start(
                                out=wt,
                                in_=wf[j0:j0 + 8].rearrange("j p o -> p j o"),
                            )
                        nc.tensor.matmul(
                            acc,
                            lhsT=oT_c[cp][:, jj, :],
                            rhs=wt[:, jj % 8, :],
                            start=(cp == 0 and jj == 0),
                            stop=last,
                            skip_group_check=True,
                        )

                    for c in range(4):
                        for b in range(B):
                            if b % 4 == 0:
                                ab4 = sb_ab.tile([128, 4, L], f16, tag="ab4")
                            qsl = qh_all[:, b, c * 128:(c + 1) * 128]
                            # --- path A: attention probabilities ---
                            ps = pp_s.tile([128, L], f32, tag="ps")
                            nc.tensor.matmul(ps, lhsT=qsl, rhs=kh_all[:, b, :])
                            ex = sb_x.tile([128, L], f32, tag="ex")
                            ssum = sb_r.tile([128, 1], f32, tag="ssum")
                            nc.scalar.activation(
                                ex, ps, func=EXP, scale=1.0 / TEMP,
                                bias=shift_s, accum_out=ssum
                            )
                            r = sb_r.tile([128, 1], f32, tag="r")
                            nc.vector.reciprocal(r, ssum)
                            nc.vector.tensor_scalar_mul(ab4[:, b % 4, :], ex, r)
                            if b % 4 == 3:
                                _store = getattr(nc, attn_store_engine)
                                _store.dma_start(
                                    out=attn_o[
                                        b - 3:b + 1, c * 128:(c + 1) * 128, :
                                    ].rearrange("b p l -> p b l"),
                                    in_=ab4,
                                )
                            # --- path B: attention output via scoresT ---
                            pst4 = pp_st.tile([128, 4, 128], f32, tag="pst4")
                            for kk in range(4):
                                nc.tensor.matmul(
                                    pst4[:, kk, :],
                                    lhsT=kh_all[:, b, kk * 128:(kk + 1) * 128],
                                    rhs=qsl,
                                )
                            eT4 = sb_e.tile([128, 4, 128], f16, tag="eT4")
                            nc.scalar.activation(eT4, pst4, func=EXP, scale=1.0 / TEMP,
                                                 bias=shift_s)
                            po = pp_o.tile([128, DV], f32, tag="po")
                            for kk in range(4):
                                nc.tensor.matmul(
                                    po, lhsT=eT4[:, kk, :], rhs=vh_all[:, b, kk, :],
                                    start=(kk == 0), stop=(kk == 3),
                                )
                            oa = sb_e.tile([128, DV], f32, tag="oa")
                            nc.vector.tensor_scalar_mul(oa, po, r)
                            pt = pp_t.tile([DV, 128], f32, tag="pt")
                            nc.tensor.transpose(pt, oa, ident)
                            ptv = pt.rearrange("p (m x) -> p m x", x=2)
                            nc.vector.tensor_copy(oT_c[c][0:64, :, b], ptv[:, :, 0])
                            nc.vector.tensor_copy(oT_c[c][64:128, :, b], ptv[:, :, 1])
                            if c >= 1:
                                for t in range(4):
                                    fc_piece(c - 1, b * 4 + t)
                    # tail: fc for the last l-chunk
                    for jj in range(64):
                        fc_piece(3, jj, last=(jj == 63))
                    nc.vector.tensor_copy(out_acc, acc)

                nc.sync.dma_start(out=out_p, in_=out_acc)

    nc.compile()
    return nc


def _get_nc():
    if "nc" not in _CACHE:
        _CACHE["nc"] = _build_nc()
    return _CACHE["nc"]


def make_in_maps(q, k, v, Wq, bq, Wk, bk, Wv, bv, Wf, bf):
    """Host-side sharding/layout prep. Returns per-core input dicts."""
    q, k, v = (np.asarray(x, np.float32) for x in (q, k, v))
    qT = np.ascontiguousarray(q.transpose(0, 2, 1)).astype(np.float16)
    kT = np.ascontiguousarray(k.transpose(0, 2, 1)).astype(np.float16)
    vTb = v.transpose(0, 2, 1).astype(np.float16)
    Wfv = np.asarray(Wf, np.float32).reshape(512, L, H, DV)
    in_maps = []
    for h in range(H):
        sl = slice(h * DK, (h + 1) * DK)
        wf_h = (
            Wfv[:, :, h, :].transpose(1, 2, 0).astype(np.float16).reshape(256, 128, 512)
        )
        in_maps.append(
            {
                "qT": qT,
                "kT": kT,
                "vT": vTb,
                "wqT": np.asarray(Wq, np.float32)[sl].T.astype(np.float16),
                "wkT": np.asarray(Wk, np.float32)[sl].T.astype(np.float16),
                "wvT": np.asarray(Wv, np.float32)[sl].T.astype(np.float16),
                "bq2": np.asarray(bq, np.float32)[sl].reshape(DK, 1).copy(),
                "bk2": np.asarray(bk, np.float32)[sl].reshape(DK, 1).copy(),
                "wf": wf_h,
            }
        )
    return in_maps


def assemble_outputs(results, bv, bf, Wf):
    """results: per-core dicts with attn_o [B,L,L] bf16 and out_p [B,512]."""
    attn_flat = np.concatenate(
        [np.asarray(results[h]["attn_o"], np.float32)[None] for h in range(H)], 0
    ).reshape(H * B, L, L)
    output = np.zeros((B, 512), np.float32)
    for h in range(H):
        output += np.asarray(results[h]["out_p"], np.float32)
    bv = np.asarray(bv, np.float32)
    if np.any(bv):
        Wfs = np.asarray(Wf, np.float32).reshape(512, L, H, DV).sum(axis=1)
        output += np.einsum("ohd,hd->o", Wfs, bv.reshape(H, DV))[None, :]
    output += np.asarray(bf, np.float32)[None, :]
    return output, attn_flat


def kernel(q, k, v, Wq, bq, Wk, bk, Wv, bv, Wf, bf):
    from concourse.bass_utils import run_bass_kernel_spmd

    nc = _get_nc()
    in_maps = make_in_maps(q, k, v, Wq, bq, Wk, bk, Wv, bv, Wf, bf)
    res = run_bass_kernel_spmd(nc, in_maps, core_ids=list(range(NCORES)))
    output, attn_flat = assemble_outputs(res.results, bv, bf, Wf)
    return output, attn_flat


# revision 19
# speedup vs baseline: 27.9574x; 1.0105x over previous
"""MultiHeadAttention TRN2 kernel — head-sharded across 8 NeuronCores.

Sharding: core h owns head h (all 16 batches). Per-head attention is
independent; the big fc weight Wf [512, 262144] is sharded along its
input dim by head (each core reads only its head's [512, 512*64] slice,
pre-permuted on host to matmul-friendly layout). The only cross-core
reduction (summing fc partials [16,512]) is done on host.

Precision: fp16 everywhere on the streaming tensors (same bytes as
bf16, 10 vs 8 mantissa bits; fp32 matmuls are 4x slower on the PE and
double the DMA); f32 softmax/PSUM. exp computed with a softmax-invariant
-2.0 shift so exp values stay well under fp16 max. Measured end-to-end:
out ~1.3e-3, attn ~1e-3 absmax-rel.

Device pipeline per core (head h), all layouts chosen so no on-device
transposes of big tensors are needed:
  - host passes qT/kT/vT (f16) pre-transposed per batch
  - qhT/khT = WqT.T @ qT -> SBUF f16 [64(dk), 512(l)]
  - vh = vT.T @ WvT (f16) -> SBUF [128(lk), 64(dv)] x4 chunks
  - scores[lq,lk] = qhT.T @ khT (f16 in, f32 PSUM); exp+rowsum in one
    ACT op; normalize; store attn (bf16, host upcasts)
  - scoresT[lk,lq] = khT.T @ qhT computed directly (swapped operands)
    instead of transposing attn; all 4 lk-chunks land in one PSUM bank
    so one ACT op does all of exp(scoresT); expT feeds attn@V as the
    stationary operand
  - out_att[lq,dv] accumulated in PSUM, normalized by the same rowsum
    reciprocal, PE-transposed to [dv,lq], and scattered into the
    fc-ready layout oT_c[(l%2)*64+d, l//2 within chunk, b]
  - fc partial: for each K-chunk j (= l-pair), matmul
    lhsT=oT_c[:,jj,:] [128,16] x rhs=Wf_perm[j] [128,512] -> acc[16,512]

DMA issue is split across both HWDGE sequencers (SP for loads, ACT for
wf/attn) and batched (qkv 2 batches, wf 8 j-chunks, attn 4 batches per
dma_start) to keep per-DMA sequencer overhead (~0.6us each) off the
critical path.
"""

import os

os.environ.setdefault("MYCRO_LOCAL_CACHE", "1")

from contextlib import ExitStack

import numpy as np
import ml_dtypes

B, L, D, H, DK, DV = 16, 512, 512, 8, 64, 64
TEMP = float(np.sqrt(DK))
NCORES = 8
BF16 = ml_dtypes.bfloat16

_CACHE = {}


def _build_nc(niter=1, ld_bufs=2, wf_bufs=7, e_bufs=4, x_bufs=3, ab_bufs=4,
              r_bufs=10, s_bufs=2, st_bufs=2, o_bufs=2, attn_store_engine="scalar",
              gate_lo=6, gate_hi=6):
    import concourse.bass as bass
    import concourse.tile as tile
    from concourse import bacc, mybir
    from concourse.masks import make_identity

    dt = mybir.dt
    f32, bf16, f16 = dt.float32, dt.bfloat16, dt.float16
    EXP = mybir.ActivationFunctionType.Exp
    EXP_SHIFT = 2.0

    nc = bacc.Bacc(
        "TRN2",
        debug=False,
        enable_asserts=False,
        target_bir_lowering=False,
        num_devices=NCORES,
    )

    qT = nc.dram_tensor("qT", [B, D, L], f16, kind="ExternalInput").ap()
    kT = nc.dram_tensor("kT", [B, D, L], f16, kind="ExternalInput").ap()
    vT = nc.dram_tensor("vT", [B, D, L], f16, kind="ExternalInput").ap()
    wqT = nc.dram_tensor("wqT", [D, DK], f16, kind="ExternalInput").ap()
    wkT = nc.dram_tensor("wkT", [D, DK], f16, kind="ExternalInput").ap()
    wvT = nc.dram_tensor("wvT", [D, DV], f16, kind="ExternalInput").ap()
    bq2 = nc.dram_tensor("bq2", [DK, 1], f32, kind="ExternalInput").ap()
    bk2 = nc.dram_tensor("bk2", [DK, 1], f32, kind="ExternalInput").ap()
    wf = nc.dram_tensor("wf", [256, 128, 512], f16, kind="ExternalInput").ap()

    attn_o = nc.dram_tensor("attn_o", [B, L, L], f16, kind="ExternalOutput").ap()
    out_p = nc.dram_tensor("out_p", [B, 512], f32, kind="ExternalOutput").ap()

    with tile.TileContext(nc) as tc:
        with ExitStack() as ctx:
            consts = ctx.enter_context(tc.tile_pool(name="consts", bufs=1))

            wq_s = consts.tile([128, 4, DK], f16, tag="wq")
            wk_s = consts.tile([128, 4, DK], f16, tag="wk")
            wv_s = consts.tile([128, 4, DV], f16, tag="wv")
            nc.sync.dma_start(out=wq_s, in_=wqT.rearrange("(c p) e -> p c e", p=128))
            nc.sync.dma_start(out=wk_s, in_=wkT.rearrange("(c p) e -> p c e", p=128))
            nc.sync.dma_start(out=wv_s, in_=wvT.rearrange("(c p) e -> p c e", p=128))
            bq_s = consts.tile([DK, 1], f32, tag="bq")
            bk_s = consts.tile([DK, 1], f32, tag="bk")
            nc.sync.dma_start(out=bq_s, in_=bq2)
            nc.sync.dma_start(out=bk_s, in_=bk2)
            ident = consts.tile([128, 128], f32, tag="ident")
            make_identity(nc, ident)
            shift_s = consts.tile([128, 1], f32, tag="shift")
            nc.vector.memset(shift_s, -EXP_SHIFT)

            qh_all = consts.tile([DK, B, L], f16, tag="qh_all")
            kh_all = consts.tile([DK, B, L], f16, tag="kh_all")
            vh_all = consts.tile([128, B, 4, DV], f16, tag="vh_all")
            # fc-ready attention output, one tile per l-chunk c so fc reads
            # of chunk c never conflict with writes of chunk c+1
            oT_c = [
                consts.tile([128, 64, B], f16, tag=f"oT{c}", name=f"oT{c}")
                for c in range(4)
            ]
            out_acc = consts.tile([B, 512], f32, tag="out_acc")

            for _it in range(niter):
                # ---------------- Phase P: projections ----------------
                with ExitStack() as pctx:
                    ld = pctx.enter_context(tc.tile_pool(name="ld", bufs=ld_bufs))
                    pp_proj = pctx.enter_context(
                        tc.tile_pool(name="pp_proj", bufs=2, space="PSUM")
                    )
                    pp_v = pctx.enter_context(
                        tc.tile_pool(name="pp_v", bufs=2, space="PSUM")
                    )
                    for b0 in range(0, B, 2):
                        qt = ld.tile([128, 2, 4, L], f16, tag="qt")
                        kt = ld.tile([128, 2, 4, L], f16, tag="kt")
                        vt = ld.tile([128, 2, 4, L], f16, tag="vt")
                        nc.sync.dma_start(
                            out=qt,
                            in_=qT[b0:b0 + 2].rearrange("b (c p) l -> p b c l", p=128),
                        )
                        nc.sync.dma_start(
                            out=kt,
                            in_=kT[b0:b0 + 2].rearrange("b (c p) l -> p b c l", p=128),
                        )
                        nc.sync.dma_start(
                            out=vt,
                            in_=vT[b0:b0 + 2].rearrange("b (c p) l -> p b c l", p=128),
                        )
                        for bb in range(2):
                            b = b0 + bb
                            psq = pp_proj.tile([DK, L], f32, tag="psq")
                            for c in range(4):
                                nc.tensor.matmul(
                                    psq, lhsT=wq_s[:, c, :], rhs=qt[:, bb, c, :],
                                    start=(c == 0), stop=(c == 3),
                                )
                            nc.vector.tensor_scalar_add(qh_all[:, b, :], psq, bq_s)
                            psk = pp_proj.tile([DK, L], f32, tag="psq")
                            for c in range(4):
                                nc.tensor.matmul(
                                    psk, lhsT=wk_s[:, c, :], rhs=kt[:, bb, c, :],
                                    start=(c == 0), stop=(c == 3),
                                )
                            nc.vector.tensor_scalar_add(kh_all[:, b, :], psk, bk_s)
                            for ls in range(4):
                                psv = pp_v.tile([128, DV], f32, tag="psv")
                                for c in range(4):
                                    nc.tensor.matmul(
                                        psv,
                                        lhsT=vt[:, bb, c, ls * 128:(ls + 1) * 128],
                                        rhs=wv_s[:, c, :],
                                        start=(c == 0), stop=(c == 3),
                                    )
                                nc.vector.tensor_copy(vh_all[:, b, ls, :], psv)

                # ---------------- Phase A: attention + fc ----------------
                with ExitStack() as actx:
                    pp_s = actx.enter_context(
                        tc.tile_pool(name="pp_s", bufs=s_bufs, space="PSUM"))
                    pp_st = actx.enter_context(
                        tc.tile_pool(name="pp_st", bufs=st_bufs, space="PSUM"))
                    pp_o = actx.enter_context(
                        tc.tile_pool(name="pp_o", bufs=o_bufs, space="PSUM"))
                    pp_t = actx.enter_context(
                        tc.tile_pool(name="pp_t", bufs=1, space="PSUM"))
                    pp_fc = actx.enter_context(
                        tc.tile_pool(name="pp_fc", bufs=1, space="PSUM"))
                    sb_e = actx.enter_context(tc.tile_pool(name="sb_e", bufs=e_bufs))
                    sb_x = actx.enter_context(tc.tile_pool(name="sb_x", bufs=x_bufs))
                    sb_ab = actx.enter_context(tc.tile_pool(name="sb_ab", bufs=ab_bufs))
                    sb_r = actx.enter_context(tc.tile_pool(name="sb_r", bufs=r_bufs))
                    wf_pool = actx.enter_context(tc.tile_pool(name="wf_pool", bufs=wf_bufs))

                    # fc accumulator: one PSUM accumulation group spanning all
                    # 256 fc matmuls. fc for chunk c-1 is software-pipelined
                    # into chunk c's attention loop (4 matmuls per batch) so
                    # PE/DMA stream fc while ACT/DVE run attention.
                    acc = pp_fc.tile([B, 512], f32, tag="acc")
                    wt = None

                    def fc_piece(cp, jj, last=False):
                        nonlocal wt
                        if jj % 8 == 0:
                            wt = wf_pool.tile([128, 8, 512], f16, tag="wt",
                                              name="wt")
                            idx = cp * 8 + jj // 8
                            if gate_lo <= idx < gate_hi:
                                # hold this prefetch until phase P's last
                                # projection lands, so it cannot steal DMA
                                # slots from the qkv loads (WAW via tiny copy)
                                nc.gpsimd.tensor_copy(
                                    wt[0:1, 0:1, 0:1], qh_all[0:1, B - 1, 0:1]
                                )
                            j0 = 64 * cp + jj
                            nc.sync.dma_start(
                                out=wt,
                                in_=wf[j0:j0 + 8].rearrange("j p o -> p j o"),
                            )
                        nc.tensor.matmul(
                            acc,
                            lhsT=oT_c[cp][:, jj, :],
                            rhs=wt[:, jj % 8, :],
                            start=(cp == 0 and jj == 0),
                            stop=last,
                            skip_group_check=True,
                        )

                    for c in range(4):
                        for b in range(B):
                            if b % 4 == 0:
                                ab4 = sb_ab.tile([128, 4, L], f16, tag="ab4")
                            qsl = qh_all[:, b, c * 128:(c + 1) * 128]
                            # --- path A: attention probabilities ---
                            ps = pp_s.tile([128, L], f32, tag="ps")
                            nc.tensor.matmul(ps, lhsT=qsl, rhs=kh_all[:, b, :])
                            ex = sb_x.tile([128, L], f32, tag="ex")
                            ssum = sb_r.tile([128, 1], f32, tag="ssum")
                            nc.scalar.activation(
                                ex, ps, func=EXP, scale=1.0 / TEMP,
                                bias=shift_s, accum_out=ssum
                            )
                            r = sb_r.tile([128, 1], f32, tag="r")
                            nc.vector.reciprocal(r, ssum)
                            nc.vector.tensor_scalar_mul(ab4[:, b % 4, :], ex, r)
                            if b % 4 == 3:
                                _store = getattr(nc, attn_store_engine)
                                _store.dma_start(
                                    out=attn_o[
                                        b - 3:b + 1, c * 128:(c + 1) * 128, :
                                    ].rearrange("b p l -> p b l"),
                                    in_=ab4,
                                )
                            # --- path B: attention output via scoresT ---
                            pst4 = pp_st.tile([128, 4, 128], f32, tag="pst4")
                            for kk in range(4):
                                nc.tensor.matmul(
                                    pst4[:, kk, :],
                                    lhsT=kh_all[:, b, kk * 128:(kk + 1) * 128],
                                    rhs=qsl,
                                )
                            eT4 = sb_e.tile([128, 4, 128], f16, tag="eT4")
                            nc.scalar.activation(eT4, pst4, func=EXP, scale=1.0 / TEMP,
                                                 bias=shift_s)
                            po = pp_o.tile([128, DV], f32, tag="po")
                            for kk in range(4):
                                nc.tensor.matmul(
                                    po, lhsT=eT4[:, kk, :], rhs=vh_all[:, b, kk, :],
                                    start=(kk == 0), stop=(kk == 3),
                                )
                            oa = sb_e.tile([128, DV], f32, tag="oa")
                            nc.vector.tensor_scalar_mul(oa, po, r)
                            pt = pp_t.tile([DV, 128], f32, tag="pt")
                            nc.tensor.transpose(pt, oa, ident)
                            ptv = pt.rearrange("p (m x) -> p m x", x=2)
                            nc.vector.tensor_copy(oT_c[c][0:64, :, b], ptv[:, :, 0])
                            nc.vector.tensor_copy(oT_c[c][64:128, :, b], ptv[:, :, 1])
                            if c >= 1:
                                for t in range(4):
                                    fc_piece(c - 1, b * 4 + t)
                    # tail: fc for the last l-chunk
                    for jj in range(64):
                        fc_piece(3, jj, last=(jj == 63))
                    nc.vector.tensor_copy(out_acc, acc)

                nc.sync.dma_start(out=out_p, in_=out_acc)

    nc.compile()
    return nc


def _get_nc():
    if "nc" not in _CACHE:
        _CACHE["nc"] = _build_nc()
    return _CACHE["nc"]


def make_in_maps(q, k, v, Wq, bq, Wk, bk, Wv, bv, Wf, bf):
    """Host-side sharding/layout prep. Returns per-core input dicts."""
    q, k, v = (np.asarray(x, np.float32) for x in (q, k, v))
    qT = np.ascontiguousarray(q.transpose(0, 2, 1)).astype(np.float16)
    kT = np.ascontiguousarray(k.transpose(0, 2, 1)).astype(np.float16)
    vTb = v.transpose(0, 2, 1).astype(np.float16)
    Wfv = np.asarray(Wf, np.float32).reshape(512, L, H, DV)
    in_maps = []
    for h in range(H):
        sl = slice(h * DK, (h + 1) * DK)
        wf_h = (
            Wfv[:, :, h, :].transpose(1, 2, 0).astype(np.float16).reshape(256, 128, 512)
        )
        in_maps.append(
            {
                "qT": qT,
                "kT": kT,
                "vT": vTb,
                "wqT": np.asarray(Wq, np.float32)[sl].T.astype(np.float16),
                "wkT": np.asarray(Wk, np.float32)[sl].T.astype(np.float16),
                "wvT": np.asarray(Wv, np.float32)[sl].T.astype(np.float16),
                "bq2": np.asarray(bq, np.float32)[sl].reshape(DK, 1).copy(),
                "bk2": np.asarray(bk, np.float32)[sl].reshape(DK, 1).copy(),
                "wf": wf_h,
            }
        )
    return in_maps


def assemble_outputs(results, bv, bf, Wf):
    """results: per-core dicts with attn_o [B,L,L] bf16 and out_p [B,512]."""
    attn_flat = np.concatenate(
        [np.asarray(results[h]["attn_o"], np.float32)[None] for h in range(H)], 0
    ).reshape(H * B, L, L)
    output = np.zeros((B, 512), np.float32)
    for h in range(H):
        output += np.asarray(results[h]["out_p"], np.float32)
    bv = np.asarray(bv, np.float32)
    if np.any(bv):
        Wfs = np.asarray(Wf, np.float32).reshape(512, L, H, DV).sum(axis=1)
        output += np.einsum("ohd,hd->o", Wfs, bv.reshape(H, DV))[None, :]
    output += np.asarray(bf, np.float32)[None, :]
    return output, attn_flat


def kernel(q, k, v, Wq, bq, Wk, bk, Wv, bv, Wf, bf):
    from concourse.bass_utils import run_bass_kernel_spmd

    nc = _get_nc()
    in_maps = make_in_maps(q, k, v, Wq, bq, Wk, bk, Wv, bv, Wf, bf)
    res = run_bass_kernel_spmd(nc, in_maps, core_ids=list(range(NCORES)))
    output, attn_flat = assemble_outputs(res.results, bv, bf, Wf)
    return output, attn_flat
